# revision 57
# baseline (speedup 1.0000x reference)
"""Trainium2 Bass kernel for a 2-layer GraphSAGE (LSTM aggregator) GNN encoder.

Math (matches the fp32 jax reference):
  L1: h1 = relu(feat @ Wself1 + LSTM16(feat[nbr]) @ Wneigh1 + b1)
  L2: h2 = h1 @ Wself2 + LSTM16(h1[nbr]) @ Wneigh2 + b2
  pool: x[g] = mean_{node in graph g} h2 ; heads: (x@Wmu+bmu, x@Wsig+bsig)

Distribution: nodes sharded across 8 cores (4096 each). The bf16 feature /
h1 tables are assembled in shared DRAM with single AllGathers (rank-major
row order == original node order) so every core can gather arbitrary
neighbor rows; per-graph sums are AllReduced and the small head matmuls run
redundantly on every core.

This runtime charges ~85ms per transferred array, ~40Mbyte/s for bytes and
~40us per *static* program instruction, so the kernel is shaped by three
rules: pack all inputs into ONE uint32 blob per core (fp8 features, 1/8
weight slabs, 16-partition-wrapped indices), keep the instruction count
tiny by running both LSTM time loops as For_i hardware loops (the body
covers all local nodes per step), and use as few collectives as possible
(feat / weights / h1 / pooled-sum AllReduce = 4 total).

On-core layout: LSTMs run feature-major (gates^T = W @ X^T) with neighbor
features delivered feature-major by dma_gather(transpose=True) from bf16
DRAM tables. Gate order is (i, f, o, g) so one sigmoid activation covers
i/f/o from the stacked PSUM gate block. LSTM h-state and all weights are
bf16; c-state and PSUM accumulation stay fp32. When every bias vector is
zero (as in this generator) the bias adds are compiled out entirely; a
has_bias variant keeps them.
"""

import numpy as np
import ml_dtypes

BF = ml_dtypes.bfloat16
F8 = ml_dtypes.float8_e4m3
F32 = np.float32

# full problem config
FULL = dict(N=32768, DEG=16, G=64, NCORE=8)
D_IN, D_FEAT, D_REP = 128, 256, 128

# weight pack: (name, full shape, np dtype); each core uploads rows
# [c*rows/8, (c+1)*rows/8) packed contiguously in the wslab section.
WPACK = [
    ("wihT1", (128, 4 * D_IN), BF),
    ("whhT1", (128, 4 * D_IN), BF),
    ("wself1", (128, D_FEAT), BF),
    ("wneigh1", (128, D_FEAT), BF),
    ("b1bc", (128, D_FEAT), F32),
    ("wihT2", (128, 2, 4 * D_FEAT), BF),
    ("whhT2", (128, 2, 4 * D_FEAT), BF),
    ("wself2", (128, 2, D_FEAT), BF),
    ("wneigh2", (128, 2, D_FEAT), BF),
    ("b2bc", (128, D_FEAT), F32),
    ("wmu", (128, 2, D_REP), BF),
    ("wsig", (128, 2, D_REP), BF),
    ("bmu", (64, D_REP), F32),
    ("bsig", (64, D_REP), F32),
]
GPERM = [0, 1, 3, 2]  # pytorch (i,f,g,o) -> kernel gate order (i,f,o,g)
BIAS_NAMES = ("b1bc", "b2bc", "bmu", "bsig")


def _wpack(has_bias):
    return WPACK if has_bias else [t for t in WPACK if t[0] not in BIAS_NAMES]


def _layout(N, DEG, G, NCORE, has_bias=False):
    NLOC = N // NCORE
    NB = NLOC // 128
    NW = NLOC // 16
    cols = {}
    c = 0
    def sec(name, width):
        nonlocal c
        cols[name] = (c, width)
        c += width
    sec("feat8", NLOC * D_IN // (128 * 4))       # fp8 features, block-major
    sec("idx_steps", DEG * NW * 2 // (128 // 16) // 4)  # [16,DEG,NW] i16
    if has_bias:
        sec("blstm1", 4)
        sec("blstm2", 8)
    sec("gl", NB // 4)
    sec("invg", 1)
    woff, wb = {}, 0
    for name, shape, dt in _wpack(has_bias):
        nbytes = int(np.prod(shape)) * np.dtype(dt).itemsize // NCORE
        assert nbytes % 4096 == 0
        woff[name] = wb
        wb += nbytes
    sec("wslab", wb // (128 * 4))
    return dict(NLOC=NLOC, NB=NB, NW=NW, WPC=wb, woff=woff, cols=cols, W=c)


def build_program(N, DEG, G, NCORE, stop_after="full", has_bias=False):
    """Build + compile the SPMD Bass program. Returns the Bacc object."""
    from contextlib import ExitStack

    import concourse.mybir as mybir
    import concourse.tile as tile
    from concourse import bacc, library_config
    from concourse.bass import ds

    f32 = mybir.dt.float32
    bf16 = mybir.dt.bfloat16
    f8e4 = mybir.dt.float8e4
    u32 = mybir.dt.uint32
    i16 = mybir.dt.int16
    i32 = mybir.dt.int32
    Sig = mybir.ActivationFunctionType.Sigmoid
    Tnh = mybir.ActivationFunctionType.Tanh
    Rlu = mybir.ActivationFunctionType.Relu
    Cpy = mybir.ActivationFunctionType.Copy
    MDT = {np.dtype(BF): bf16, np.dtype(F32): f32}

    L = _layout(N, DEG, G, NCORE, has_bias)
    NLOC, NB, NW, WPC = L["NLOC"], L["NB"], L["NW"], L["WPC"]
    cols, woff, W = L["cols"], L["woff"], L["W"]
    WR = WPC // 512                    # weight-slab rows of 128 f32
    L1G = 1024                         # L1 node-group size
    NG1 = NLOC // L1G
    L2G = 512                          # L2 node-group size
    shared = "Shared" if NCORE > 4 else "Local"
    rg = [list(range(NCORE))]

    nc = bacc.Bacc("TRN2", target_bir_lowering=False, debug=False,
                   num_devices=NCORE)

    blob = nc.dram_tensor("blob", [128, W], u32, kind="ExternalInput")

    def bsec(name, dtype, rows=slice(None)):
        c0, w = cols[name]
        return blob[rows, c0:c0 + w].bitcast(dtype)

    if stop_after == "full":
        out2 = nc.dram_tensor("out2", [2, G, D_REP], bf16, kind="ExternalOutput")
    elif stop_after == "l2":
        dbg_h2 = nc.dram_tensor("dbg_h2", [NLOC, D_FEAT], bf16,
                                kind="ExternalOutput")
        dbg_pr = nc.dram_tensor("dbg_pr", [128, 2, G], f32,
                                kind="ExternalOutput")
    else:
        dbg_h1 = nc.dram_tensor("dbg_h1", [N, D_FEAT], bf16,
                                kind="ExternalOutput")

    feat_bf = nc.dram_tensor("feat_bf", [NLOC, D_IN], bf16, kind="Internal")
    tab_feat = nc.dram_tensor("tab_feat", [N, D_IN], bf16, kind="Internal",
                              addr_space=shared)
    wstg = nc.dram_tensor("wstg", [WR, 128], f32, kind="Internal")
    wfull = nc.dram_tensor("wfull", [NCORE, WR, 128], f32, kind="Internal",
                           addr_space=shared)
    h1_shard = nc.dram_tensor("h1_shard", [NLOC, D_FEAT], bf16, kind="Internal")
    h1_full = nc.dram_tensor("h1_full", [N, D_FEAT], bf16, kind="Internal",
                             addr_space=shared)
    pr_in = nc.dram_tensor("pr_in", [128, 2, G], f32, kind="Internal")
    pr_out = nc.dram_tensor("pr_out", [128, 2, G], f32, kind="Internal",
                            addr_space=shared)

    nc.gpsimd.load_library(library_config.mlp)

    with tile.TileContext(nc) as tc, ExitStack() as ctx:
        consts = ctx.enter_context(tc.tile_pool(name="consts", bufs=1))

        # ---- weight slabs: blob -> staging -> one AllGather -> SBUF
        nc.sync.dma_start(out=wstg[:, :], in_=bsec("wslab", f32))
        nc.gpsimd.collective_compute(
            "AllGather", mybir.AluOpType.bypass, replica_groups=rg,
            ins=[wstg[:, :]], outs=[wfull[:, :, :]])

        def gload(name):
            # one strided DMA reads all 8 rank-chunks of this tensor from
            # the gathered slab in rank-major order (== the tile row order)
            shape = next(s for n, s, _ in WPACK if n == name)
            npdt = next(d for n, _, d in WPACK if n == name)
            dt = MDT[np.dtype(npdt)]
            r0 = woff[name] // 512
            nr = int(np.prod(shape)) * np.dtype(npdt).itemsize // NCORE // 512
            t = consts.tile(list(shape), dt, tag=name)
            nc.sync.dma_start(out=t,
                              in_=wfull[:, r0:r0 + nr, :].bitcast(dt))
            return t

        # ---- gather indices: replicate 16-row wrap to the 8 gpsimd
        # stripes with a doubling ladder
        idxs_sb = consts.tile([128, DEG, NW], i16, tag="idxs")
        idxl_sb = consts.tile([128, NW], i16, tag="idxl")
        nc.gpsimd.iota(idxl_sb[0:16, :], pattern=[[16, NW]], base=0,
                       channel_multiplier=1)
        nc.sync.dma_start(out=idxs_sb[0:16, :, :], in_=bsec("idx_steps", i16))
        for k in (16, 32, 64):
            nc.sync.dma_start(out=idxs_sb[k:2 * k, :, :],
                              in_=idxs_sb[0:k, :, :])
            nc.sync.dma_start(out=idxl_sb[k:2 * k, :], in_=idxl_sb[0:k, :])
        idx_t = consts.tile([128, 1, NW], i16, tag="idx_t")

        def cload(sec, shape, dtype, tag, rows=slice(None)):
            t = consts.tile(shape, dtype, tag=tag)
            nc.sync.dma_start(out=t, in_=bsec(sec, dtype, rows))
            return t

        gl8_sb = cload("gl", [128, NB], mybir.dt.int8, "gl8")
        gl_sb = consts.tile([128, NB], f32, tag="gl")
        nc.vector.tensor_copy(gl_sb, gl8_sb)
        inv_sb = cload("invg", [G, 1], f32, "invg", rows=slice(0, G))
        if has_bias:
            blstm1_sb = cload("blstm1", [128, 4], f32, "blstm1")
            blstm2_sb = cload("blstm2", [128, 8], f32, "blstm2")
            b1bc_sb = gload("b1bc")
            b2bc_sb = gload("b2bc")
            bmu_sb = gload("bmu")
            bsig_sb = gload("bsig")

        wihT1_sb = gload("wihT1")
        whhT1_sb = gload("whhT1")
        wself1_sb = gload("wself1")
        wneigh1_sb = gload("wneigh1")
        wihT2_sb = gload("wihT2")
        whhT2_sb = gload("whhT2")
        wself2_sb = gload("wself2")
        wneigh2_sb = gload("wneigh2")
        wmu_sb = gload("wmu")
        wsig_sb = gload("wsig")

        # column-index iota row, for building per-block pool one-hots
        # (values 0..G-1 are exact in f32)
        iotaF = consts.tile([128, G], f32, tag="iotaF")
        nc.gpsimd.iota(iotaF, pattern=[[1, G]], base=0, channel_multiplier=0,
                       allow_small_or_imprecise_dtypes=True)

        # fp8 shard -> bf16 local table -> single AllGather (rank-major ==
        # original node order, so neighbor ids index the tables directly)
        with tc.tile_pool(name="cvt", bufs=2) as cvt:
            t8 = cvt.tile([128, NB * D_IN], f8e4, tag="t8")
            nc.sync.dma_start(out=t8, in_=bsec("feat8", f8e4))
            tb = cvt.tile([128, NB * D_IN], bf16, tag="tb")
            nc.vector.tensor_copy(tb, t8)
            for blk in range(NB):
                nc.sync.dma_start(
                    out=feat_bf[blk * 128:(blk + 1) * 128, :],
                    in_=tb[:, blk * 128:(blk + 1) * 128])
        nc.gpsimd.collective_compute(
            "AllGather", mybir.AluOpType.bypass, replica_groups=rg,
            ins=[feat_bf[:, :]], outs=[tab_feat[:, :]])

        snp = ctx.enter_context(tc.tile_pool(name="snp", bufs=3))

        # ================= Layer 1 =================
        with tc.tile_pool(name="st1", bufs=1) as st1, \
             tc.tile_pool(name="gt1", bufs=1) as gtp, \
             tc.tile_pool(name="xg1p", bufs=1) as xgp:
            hN1 = st1.tile([128, NLOC], bf16, tag="hN1")
            cN1 = st1.tile([128, NLOC], f32, tag="cN1")
            nc.vector.memset(hN1, 0.0)
            nc.vector.memset(cN1, 0.0)
            featT = st1.tile([128, 1, NLOC], bf16, tag="featT")
            nc.gpsimd.dma_gather(featT[:], feat_bf[:], idxl_sb[:, :],
                                 NLOC, NLOC, D_IN, transpose=True,
                                 single_packet=False)

            with tc.tile_pool(name="psl1", bufs=1, space="PSUM") as psl:
                with tc.For_i(0, DEG) as t:
                    nc.sync.dma_start(out=idx_t,
                                      in_=idxs_sb[:, ds(t, 1), :])
                    xg = xgp.tile([128, 1, NLOC], bf16, tag="xg1")
                    nc.gpsimd.dma_gather(
                        xg[:], tab_feat[:], idx_t[:, 0, :],
                        NLOC, NLOC, D_IN, transpose=True,
                        single_packet=False)
                    with tc.For_i(0, NLOC, 512) as go:
                        gsl = ds(go, 512)
                        ps = psl.tile([128, 4, 512], f32, tag="ps1")
                        for q in range(4):
                            o = ps[:, q, :]
                            nc.tensor.matmul(
                                o, wihT1_sb[:, q * 128:(q + 1) * 128],
                                xg[:, 0, gsl], start=True, stop=False)
                            nc.tensor.matmul(
                                o, whhT1_sb[:, q * 128:(q + 1) * 128],
                                hN1[:, gsl], start=False, stop=True)
                        ifo = gtp.tile([128, 3, 512], f32, tag="ifo1")
                        gg = gtp.tile([128, 1, 512], f32, tag="gg1")
                        if has_bias:
                            for q, fn in ((0, Sig), (1, Sig), (2, Sig)):
                                nc.scalar.activation(
                                    ifo[:, q, :], ps[:, q, :], fn,
                                    bias=blstm1_sb[:, q:q + 1])
                            nc.scalar.activation(gg[:, 0, :], ps[:, 3, :],
                                                 Tnh, bias=blstm1_sb[:, 3:4])
                        else:
                            nc.scalar.activation(ifo, ps[:, 0:3, :], Sig)
                            nc.scalar.activation(gg, ps[:, 3:4, :], Tnh)
                        t0 = gtp.tile([128, 512], f32, tag="t01")
                        nc.vector.tensor_mul(t0, ifo[:, 0, :], gg[:, 0, :])
                        nc.vector.tensor_mul(cN1[:, gsl], cN1[:, gsl],
                                             ifo[:, 1, :])
                        nc.vector.tensor_add(cN1[:, gsl], cN1[:, gsl], t0)
                        tch = gtp.tile([128, 512], f32, tag="tch1")
                        nc.scalar.activation(tch, cN1[:, gsl], Tnh)
                        nc.vector.tensor_mul(hN1[:, gsl], ifo[:, 2, :], tch)

            # self/neigh + relu -> h1_shard; then one AllGather. The node
            # slices are staged through fixed tiles so the matmul lhsT stays
            # register-offset-free inside the loop.
            with tc.tile_pool(name="psm1", bufs=2, space="PSUM") as psm, \
                 tc.tile_pool(name="tmp1", bufs=1) as tmpp:
                with tc.For_i(0, NLOC, 128) as bo:
                    tf = tmpp.tile([128, 128], bf16, tag="tf")
                    nc.sync.dma_start(out=tf, in_=featT[:, 0, ds(bo, 128)])
                    th = tmpp.tile([128, 128], bf16, tag="th")
                    nc.sync.dma_start(out=th, in_=hN1[:, ds(bo, 128)])
                    ps = psm.tile([128, D_FEAT], f32, tag="psm1")
                    nc.tensor.matmul(ps, tf, wself1_sb[:, :],
                                     start=True, stop=False)
                    nc.tensor.matmul(ps, th, wneigh1_sb[:, :],
                                     start=False, stop=True)
                    h1b = snp.tile([128, D_FEAT], bf16, tag="sn1b")
                    if has_bias:
                        tmp = snp.tile([128, D_FEAT], f32, tag="sn1t")
                        nc.vector.tensor_add(tmp, ps, b1bc_sb)
                        nc.scalar.activation(h1b, tmp, Rlu)
                    else:
                        nc.scalar.activation(h1b, ps, Rlu)
                    nc.sync.dma_start(out=h1_shard[ds(bo, 128), :], in_=h1b)
            nc.gpsimd.collective_compute(
                "AllGather", mybir.AluOpType.bypass, replica_groups=rg,
                ins=[h1_shard[:, :]], outs=[h1_full[:, :]])

        import concourse.mybir as _mb

        if stop_after == "l1":
            with tc.tile_pool(name="dbgp", bufs=2) as dbgp:
                for blk in range(N // 128):
                    dt_ = dbgp.tile([128, D_FEAT], bf16, tag="dbg")
                    nc.sync.dma_start(
                        out=dt_, in_=h1_full[blk * 128:(blk + 1) * 128, :])
                    nc.sync.dma_start(
                        out=dbg_h1[blk * 128:(blk + 1) * 128, :], in_=dt_)
        else:
            # ================= Layer 2 =================
            with tc.tile_pool(name="st2", bufs=1) as st2, \
                 tc.tile_pool(name="gt2", bufs=1) as gtp, \
                 tc.tile_pool(name="xg2p", bufs=1) as xgp:
                hN2 = st2.tile([128, 2, NLOC], bf16, tag="hN2")
                cN2 = st2.tile([128, 2, NLOC], f32, tag="cN2")
                nc.vector.memset(hN2, 0.0)
                nc.vector.memset(cN2, 0.0)
                h1T = st2.tile([128, 2, NLOC], bf16, tag="h1T")
                nc.gpsimd.dma_gather(h1T[:], h1_shard[:], idxl_sb[:, :],
                                     NLOC, NLOC, D_FEAT, transpose=True,
                                     single_packet=False)

                with tc.tile_pool(name="psl2", bufs=1, space="PSUM") as psl:
                    with tc.For_i(0, DEG) as t:
                        nc.sync.dma_start(out=idx_t,
                                          in_=idxs_sb[:, ds(t, 1), :])
                        xg = xgp.tile([128, 2, NLOC], bf16, tag="xg2")
                        nc.gpsimd.dma_gather(
                            xg[:], h1_full[:], idx_t[:, 0, :],
                            NLOC, NLOC, D_FEAT, transpose=True,
                            single_packet=False)
                        with tc.For_i(0, NLOC, L2G) as so:
                            nsl = ds(so, L2G)
                            ps = psl.tile([128, 8, L2G], f32, tag="ps2")
                            for q in range(4):
                                for mb in range(2):
                                    o = ps[:, 2 * q + mb, :]
                                    wsl = slice(q * 256 + mb * 128,
                                                q * 256 + (mb + 1) * 128)
                                    for kb in range(2):
                                        nc.tensor.matmul(
                                            o, wihT2_sb[:, kb, wsl],
                                            xg[:, kb, nsl],
                                            start=(kb == 0), stop=False)
                                    for kb in range(2):
                                        nc.tensor.matmul(
                                            o, whhT2_sb[:, kb, wsl],
                                            hN2[:, kb, nsl],
                                            start=False, stop=(kb == 1))
                            ifo = gtp.tile([128, 6, L2G], f32, tag="ifo2")
                            gg2 = gtp.tile([128, 2, L2G], f32, tag="gg2")
                            if has_bias:
                                for q in range(3):
                                    for mb in range(2):
                                        nc.scalar.activation(
                                            ifo[:, 2 * q + mb, :],
                                            ps[:, 2 * q + mb, :], Sig,
                                            bias=blstm2_sb[:, 2 * q + mb:
                                                           2 * q + mb + 1])
                                for mb in range(2):
                                    nc.scalar.activation(
                                        gg2[:, mb, :], ps[:, 6 + mb, :], Tnh,
                                        bias=blstm2_sb[:, 6 + mb:7 + mb])
                            else:
                                nc.scalar.activation(ifo, ps[:, 0:6, :], Sig)
                                nc.scalar.activation(gg2, ps[:, 6:8, :], Tnh)
                            t0 = gtp.tile([128, 2, L2G], f32, tag="t02")
                            nc.vector.tensor_mul(t0, ifo[:, 0:2, :], gg2)
                            nc.vector.tensor_mul(cN2[:, :, nsl],
                                                 cN2[:, :, nsl],
                                                 ifo[:, 2:4, :])
                            nc.vector.tensor_add(cN2[:, :, nsl],
                                                 cN2[:, :, nsl], t0)
                            tch = gtp.tile([128, 2, L2G], f32, tag="tch2")
                            nc.scalar.activation(tch, cN2[:, :, nsl], Tnh)
                            nc.vector.tensor_mul(hN2[:, :, nsl],
                                                 ifo[:, 4:6, :], tch)

                # L2 self/neigh + pooling, as a hardware loop: node slices
                # staged into fixed tiles (matmul lhsT can't take register
                # offsets); the pool one-hots are pre-built in a 128-padded
                # bank so ds(node_offset, G) indexes them; the pool PSUM
                # accumulation group is opened/closed by K=1 zero matmuls.
                ohp = st2.tile([128, NB * 128], bf16, tag="ohp")
                for blk in range(NB):
                    nc.vector.tensor_scalar(
                        ohp[:, blk * 128:blk * 128 + G], iotaF,
                        gl_sb[:, blk:blk + 1], None, _mb.AluOpType.is_equal)
                with tc.tile_pool(name="psm2", bufs=2, space="PSUM") as psm, \
                     tc.tile_pool(name="pspool", bufs=2, space="PSUM") as psp, \
                     tc.tile_pool(name="pshead", bufs=2, space="PSUM") as psh, \
                     tc.tile_pool(name="tmp2", bufs=1) as tmpp:
                    pool_ps = [psp.tile([128, G], f32, tag=f"pool{mh}",
                                        name=f"pool_ps{mh}")
                               for mh in range(2)]
                    zrow = snp.tile([1, 128], bf16, tag="zrow")
                    nc.vector.memset(zrow, 0.0)
                    for mh in range(2):
                        nc.tensor.matmul(pool_ps[mh], zrow[0:1, :],
                                         zrow[0:1, 0:G],
                                         start=True, stop=False,
                                         skip_group_check=True)
                    with tc.For_i(0, NLOC, 128) as bo:
                        t1 = tmpp.tile([128, 2, 128], bf16, tag="t1")
                        nc.sync.dma_start(out=t1, in_=h1T[:, :, ds(bo, 128)])
                        t2 = tmpp.tile([128, 2, 128], bf16, tag="t2")
                        nc.sync.dma_start(out=t2, in_=hN2[:, :, ds(bo, 128)])
                        ps = psm.tile([128, D_FEAT], f32, tag="psm2")
                        for kb in range(2):
                            nc.tensor.matmul(ps, t1[:, kb, :],
                                             wself2_sb[:, kb, :],
                                             start=(kb == 0), stop=False)
                        for kb in range(2):
                            nc.tensor.matmul(ps, t2[:, kb, :],
                                             wneigh2_sb[:, kb, :],
                                             start=False, stop=(kb == 1))
                        h2sb = snp.tile([128, D_FEAT], bf16, tag="h2sb")
                        if has_bias:
                            nc.vector.tensor_add(h2sb, ps, b2bc_sb)
                        else:
                            nc.vector.tensor_copy(h2sb, ps)
                        if stop_after == "l2":
                            nc.sync.dma_start(out=dbg_h2[ds(bo, 128), :],
                                              in_=h2sb)
                        for mh in range(2):
                            nc.tensor.matmul(
                                pool_ps[mh], h2sb[:, mh * 128:(mh + 1) * 128],
                                ohp[:, ds(bo, G)],
                                start=False, stop=False,
                                skip_group_check=True)
                    for mh in range(2):
                        nc.tensor.matmul(pool_ps[mh], zrow[0:1, :],
                                         zrow[0:1, 0:G],
                                         start=False, stop=True,
                                         skip_group_check=True)
                    prcp = snp.tile([128, 2, G], f32, tag="prcp")
                    for mh in range(2):
                        nc.vector.tensor_copy(prcp[:, mh, :], pool_ps[mh])
                    nc.sync.dma_start(out=pr_in[:, :, :], in_=prcp)
                    nc.gpsimd.collective_compute(
                        "AllReduce", _mb.AluOpType.add, replica_groups=rg,
                        ins=[pr_in[:]], outs=[pr_out[:]])
                    prx = snp.tile([128, 2, G], f32, tag="prx")
                    nc.sync.dma_start(out=prx, in_=pr_out[:, :, :])
                    if stop_after == "l2":
                        nc.sync.dma_start(out=dbg_pr[:, :, :], in_=prx)
                    else:
                        prb = snp.tile([128, 2, G], bf16, tag="prb")
                        nc.vector.tensor_copy(prb, prx)
                        for hd, (wsb, bsb) in enumerate((
                                (wmu_sb, "bmu"), (wsig_sb, "bsig"))):
                            ph = psh.tile([G, D_REP], f32, tag="ph")
                            for kb in range(2):
                                nc.tensor.matmul(ph, prb[:, kb, :],
                                                 wsb[:, kb, :],
                                                 start=(kb == 0),
                                                 stop=(kb == 1))
                            # mean = sum * (1/cnt), per graph row
                            ores = snp.tile([G, D_REP], bf16, tag="ores")
                            if has_bias:
                                phs = snp.tile([G, D_REP], f32, tag="phs")
                                nc.scalar.activation(phs, ph, Cpy,
                                                     scale=inv_sb[:, 0:1])
                                nc.vector.tensor_add(
                                    ores, phs,
                                    bmu_sb if bsb == "bmu" else bsig_sb)
                            else:
                                nc.scalar.activation(ores, ph, Cpy,
                                                     scale=inv_sb[:, 0:1])
                            nc.sync.dma_start(out=out2[hd, :, :], in_=ores)


    nc.compile()
    return nc


def make_inmaps(inputs, N, DEG, G, NCORE):
    """Host-side preprocessing: pack the full inputs into per-core blobs."""
    hb = _has_bias(inputs)
    L = _layout(N, DEG, G, NCORE, hb)
    NLOC, NB, NW, WPC = L["NLOC"], L["NB"], L["NW"], L["WPC"]
    cols, W = L["cols"], L["W"]

    feat = np.asarray(inputs["in_feat"], dtype=F32)
    nbr = np.asarray(inputs["neighbors"], dtype=np.int64)
    n2g = np.asarray(inputs["node2graph"], dtype=np.int64)

    def A(name):
        return np.asarray(inputs[name], dtype=F32)

    def wrap16(ids):
        # ids [n] -> [16, n//16] int16 wrap -> [128, n//8 bytes] row split
        # matching the on-device stripe-replication DMA's flat streaming.
        w = np.ascontiguousarray(ids.reshape(-1, 16).T.astype(np.int16))
        return w.view(np.uint8).reshape(128, -1)

    def gperm(wT, ngate_el):
        # reorder the gate axis (pytorch i,f,g,o -> i,f,o,g) of a packed
        # [..., 4*ngate_el] last axis
        s = wT.shape
        w = wT.reshape(s[:-1] + (4, ngate_el))[..., GPERM, :]
        return np.ascontiguousarray(w.reshape(s))

    full = {
        "wihT1": gperm(np.ascontiguousarray(A("w_ih1").T), 128).astype(BF),
        "whhT1": gperm(np.ascontiguousarray(A("w_hh1").T), 128).astype(BF),
        "wself1": A("w_self1").astype(BF),
        "wneigh1": A("w_neigh1").astype(BF),
        "b1bc": np.tile(A("b1")[None, :], (128, 1)).astype(F32),
        "wihT2": gperm(np.ascontiguousarray(
            A("w_ih2").T.reshape(2, 128, 4 * D_FEAT).transpose(1, 0, 2)),
            D_FEAT).astype(BF),
        "whhT2": gperm(np.ascontiguousarray(
            A("w_hh2").T.reshape(2, 128, 4 * D_FEAT).transpose(1, 0, 2)),
            D_FEAT).astype(BF),
        "wself2": np.ascontiguousarray(
            A("w_self2").reshape(2, 128, D_FEAT).transpose(1, 0, 2)).astype(BF),
        "wneigh2": np.ascontiguousarray(
            A("w_neigh2").reshape(2, 128, D_FEAT).transpose(1, 0, 2)).astype(BF),
        "b2bc": np.tile(A("b2")[None, :], (128, 1)).astype(F32),
        "wmu": np.ascontiguousarray(
            A("w_mu").reshape(2, 128, D_REP).transpose(1, 0, 2)).astype(BF),
        "wsig": np.ascontiguousarray(
            A("w_sigma").reshape(2, 128, D_REP).transpose(1, 0, 2)).astype(BF),
        "bmu": np.tile(A("b_mu")[None, :], (G, 1)).astype(F32),
        "bsig": np.tile(A("b_sigma")[None, :], (G, 1)).astype(F32),
    }

    blstm1 = np.ascontiguousarray(
        A("b_lstm1").reshape(4, 128)[GPERM].T).astype(F32)      # [128, 4]
    blstm2 = np.ascontiguousarray(
        A("b_lstm2").reshape(4, 2, 128)[GPERM].transpose(2, 0, 1)
        .reshape(128, 8)).astype(F32)

    cnt = np.bincount(n2g, minlength=G).astype(F32)
    inv = (1.0 / np.maximum(cnt, 1.0))[:, None].astype(F32)     # [G, 1]

    def put(bb, sec, rows, arr8):
        c0, w = cols[sec]
        bb[rows, c0 * 4:c0 * 4 + arr8.shape[-1]] = arr8

    in_maps = []
    for c in range(NCORE):
        base = c * NLOC
        blob = np.zeros((128, W), np.uint32)
        bb = blob.view(np.uint8).reshape(128, W * 4)

        f8 = feat[base:base + NLOC].astype(F8)
        f8 = f8.reshape(NB, 128, D_IN).transpose(1, 0, 2).reshape(128, NB * D_IN)
        put(bb, "feat8", slice(None), f8.view(np.uint8))

        ids = nbr[base:base + NLOC, :]  # [NLOC, DEG]
        w16 = np.ascontiguousarray(
            ids.reshape(NW, 16, DEG).transpose(1, 2, 0).astype(np.int16))
        steps8 = w16.view(np.uint8).reshape(128, -1)
        put(bb, "idx_steps", slice(None), steps8)
        if hb:
            put(bb, "blstm1", slice(None), blstm1.view(np.uint8))
            put(bb, "blstm2", slice(None), blstm2.view(np.uint8))
        gl = np.ascontiguousarray(
            n2g[base:base + NLOC].reshape(NB, 128).T.astype(np.int8))
        put(bb, "gl", slice(None), gl.view(np.uint8))
        put(bb, "invg", slice(0, G), inv.view(np.uint8))

        slab = b"".join(
            np.ascontiguousarray(
                full[name][(full[name].shape[0] // NCORE) * c:
                           (full[name].shape[0] // NCORE) * (c + 1)]
            ).tobytes()
            for name, _, _ in _wpack(hb))
        put(bb, "wslab", slice(None),
            np.frombuffer(slab, np.uint8).reshape(128, WPC // 128))

        in_maps.append({"blob": blob})
    return in_maps


def _has_bias(inputs):
    return any(
        np.any(np.asarray(inputs[k]))
        for k in ("b_lstm1", "b_lstm2", "b1", "b2", "b_mu", "b_sigma"))


_PROG = None
_PROG_BIAS = None


def kernel(**inputs):
    global _PROG, _PROG_BIAS
    from concourse.bass_utils import run_bass_kernel_spmd

    cfg = FULL
    hb = _has_bias(inputs)
    if _PROG is None or _PROG_BIAS != hb:
        _PROG = build_program(**cfg, has_bias=hb)
        _PROG_BIAS = hb
    in_maps = make_inmaps(inputs, **cfg)
    res = run_bass_kernel_spmd(_PROG, in_maps, core_ids=list(range(cfg["NCORE"])))
    out = np.asarray(res.results[0]["out2"], dtype=np.float32)
    return (out[0], out[1])


# revision 58
# speedup vs baseline: 1.0731x; 1.0731x over previous
"""Trainium2 Bass kernel for a 2-layer GraphSAGE (LSTM aggregator) GNN encoder.

Math (matches the fp32 jax reference):
  L1: h1 = relu(feat @ Wself1 + LSTM16(feat[nbr]) @ Wneigh1 + b1)
  L2: h2 = h1 @ Wself2 + LSTM16(h1[nbr]) @ Wneigh2 + b2
  pool: x[g] = mean_{node in graph g} h2 ; heads: (x@Wmu+bmu, x@Wsig+bsig)

Distribution: nodes sharded across 8 cores (4096 each). The bf16 feature /
h1 tables are assembled in shared DRAM with single AllGathers (rank-major
row order == original node order) so every core can gather arbitrary
neighbor rows; per-graph sums are AllReduced and the small head matmuls run
redundantly on every core.

This runtime charges ~85ms per transferred array, ~40Mbyte/s for bytes and
~40us per *static* program instruction, so the kernel is shaped by three
rules: pack all inputs into ONE uint32 blob per core (fp8 features, 1/8
weight slabs, 16-partition-wrapped indices), keep the instruction count
tiny by running both LSTM time loops as For_i hardware loops (the body
covers all local nodes per step), and use as few collectives as possible
(feat / weights / h1 / pooled-sum AllReduce = 4 total).

On-core layout: LSTMs run feature-major (gates^T = W @ X^T) with neighbor
features delivered feature-major by dma_gather(transpose=True) from bf16
DRAM tables. Gate order is (i, f, o, g) so one sigmoid activation covers
i/f/o from the stacked PSUM gate block. LSTM h-state and all weights are
bf16; c-state and PSUM accumulation stay fp32. When every bias vector is
zero (as in this generator) the bias adds are compiled out entirely; a
has_bias variant keeps them.
"""

import numpy as np
import ml_dtypes

BF = ml_dtypes.bfloat16
F8 = ml_dtypes.float8_e4m3
F32 = np.float32

# full problem config
FULL = dict(N=32768, DEG=16, G=64, NCORE=8)
D_IN, D_FEAT, D_REP = 128, 256, 128

# weight pack: (name, full shape, np dtype); each core uploads rows
# [c*rows/8, (c+1)*rows/8) packed contiguously in the wslab section.
WPACK = [
    ("wihT1", (128, 4 * D_IN), F8),
    ("whhT1", (128, 4 * D_IN), F8),
    ("wself1", (128, D_FEAT), BF),
    ("wneigh1", (128, D_FEAT), BF),
    ("b1bc", (128, D_FEAT), F32),
    ("wihT2", (128, 2, 4 * D_FEAT), F8),
    ("whhT2", (128, 2, 4 * D_FEAT), F8),
    ("wself2", (128, 2, D_FEAT), BF),
    ("wneigh2", (128, 2, D_FEAT), BF),
    ("b2bc", (128, D_FEAT), F32),
    ("wmu", (128, 2, D_REP), BF),
    ("wsig", (128, 2, D_REP), BF),
    ("bmu", (64, D_REP), F32),
    ("bsig", (64, D_REP), F32),
]
GPERM = [0, 1, 3, 2]  # pytorch (i,f,g,o) -> kernel gate order (i,f,o,g)
BIAS_NAMES = ("b1bc", "b2bc", "bmu", "bsig")


def _wpack(has_bias):
    return WPACK if has_bias else [t for t in WPACK if t[0] not in BIAS_NAMES]


def _layout(N, DEG, G, NCORE, has_bias=False):
    NLOC = N // NCORE
    NB = NLOC // 128
    NW = NLOC // 16
    cols = {}
    c = 0
    def sec(name, width):
        nonlocal c
        cols[name] = (c, width)
        c += width
    sec("feat8", NLOC * D_IN // (128 * 4))       # fp8 features, block-major
    sec("idx_steps", DEG * NW * 2 // (128 // 16) // 4)  # [16,DEG,NW] i16
    if has_bias:
        sec("blstm1", 4)
        sec("blstm2", 8)
    sec("gl", NB // 4)
    sec("invg", 1)
    woff, wb = {}, 0
    for name, shape, dt in _wpack(has_bias):
        nbytes = int(np.prod(shape)) * np.dtype(dt).itemsize // NCORE
        assert nbytes % 4096 == 0
        woff[name] = wb
        wb += nbytes
    sec("wslab", wb // (128 * 4))
    return dict(NLOC=NLOC, NB=NB, NW=NW, WPC=wb, woff=woff, cols=cols, W=c)


def build_program(N, DEG, G, NCORE, stop_after="full", has_bias=False):
    """Build + compile the SPMD Bass program. Returns the Bacc object."""
    from contextlib import ExitStack

    import concourse.mybir as mybir
    import concourse.tile as tile
    from concourse import bacc, library_config
    from concourse.bass import ds

    f32 = mybir.dt.float32
    bf16 = mybir.dt.bfloat16
    f8e4 = mybir.dt.float8e4
    u32 = mybir.dt.uint32
    i16 = mybir.dt.int16
    i32 = mybir.dt.int32
    Sig = mybir.ActivationFunctionType.Sigmoid
    Tnh = mybir.ActivationFunctionType.Tanh
    Rlu = mybir.ActivationFunctionType.Relu
    Cpy = mybir.ActivationFunctionType.Copy
    MDT = {np.dtype(BF): bf16, np.dtype(F32): f32, np.dtype(F8): f8e4}

    L = _layout(N, DEG, G, NCORE, has_bias)
    NLOC, NB, NW, WPC = L["NLOC"], L["NB"], L["NW"], L["WPC"]
    cols, woff, W = L["cols"], L["woff"], L["W"]
    WR = WPC // 512                    # weight-slab rows of 128 f32
    L1G = 1024                         # L1 node-group size
    NG1 = NLOC // L1G
    L2G = 512                          # L2 node-group size
    shared = "Shared" if NCORE > 4 else "Local"
    rg = [list(range(NCORE))]

    nc = bacc.Bacc("TRN2", target_bir_lowering=False, debug=False,
                   num_devices=NCORE)

    blob = nc.dram_tensor("blob", [128, W], u32, kind="ExternalInput")

    def bsec(name, dtype, rows=slice(None)):
        c0, w = cols[name]
        return blob[rows, c0:c0 + w].bitcast(dtype)

    if stop_after == "full":
        out2 = nc.dram_tensor("out2", [2, G, D_REP], bf16, kind="ExternalOutput")
    elif stop_after == "l2":
        dbg_h2 = nc.dram_tensor("dbg_h2", [NLOC, D_FEAT], bf16,
                                kind="ExternalOutput")
        dbg_pr = nc.dram_tensor("dbg_pr", [128, 2, G], f32,
                                kind="ExternalOutput")
    else:
        dbg_h1 = nc.dram_tensor("dbg_h1", [N, D_FEAT], bf16,
                                kind="ExternalOutput")

    feat_bf = nc.dram_tensor("feat_bf", [NLOC, D_IN], bf16, kind="Internal")
    tab_feat = nc.dram_tensor("tab_feat", [N, D_IN], bf16, kind="Internal",
                              addr_space=shared)
    wstg = nc.dram_tensor("wstg", [WR, 128], f32, kind="Internal")
    wfull = nc.dram_tensor("wfull", [NCORE, WR, 128], f32, kind="Internal",
                           addr_space=shared)
    h1_shard = nc.dram_tensor("h1_shard", [NLOC, D_FEAT], bf16, kind="Internal")
    h1_full = nc.dram_tensor("h1_full", [N, D_FEAT], bf16, kind="Internal",
                             addr_space=shared)
    pr_in = nc.dram_tensor("pr_in", [128, 2, G], f32, kind="Internal")
    pr_out = nc.dram_tensor("pr_out", [128, 2, G], f32, kind="Internal",
                            addr_space=shared)

    nc.gpsimd.load_library(library_config.mlp)

    with tile.TileContext(nc) as tc, ExitStack() as ctx:
        consts = ctx.enter_context(tc.tile_pool(name="consts", bufs=1))

        # ---- weight slabs: blob -> staging -> one AllGather -> SBUF
        nc.sync.dma_start(out=wstg[:, :], in_=bsec("wslab", f32))
        nc.gpsimd.collective_compute(
            "AllGather", mybir.AluOpType.bypass, replica_groups=rg,
            ins=[wstg[:, :]], outs=[wfull[:, :, :]])

        def gload(name):
            # one strided DMA reads all 8 rank-chunks of this tensor from
            # the gathered slab in rank-major order (== the tile row order)
            shape = next(s for n, s, _ in WPACK if n == name)
            npdt = next(d for n, _, d in WPACK if n == name)
            dt = MDT[np.dtype(npdt)]
            r0 = woff[name] // 512
            nr = int(np.prod(shape)) * np.dtype(npdt).itemsize // NCORE // 512
            t = consts.tile(list(shape), dt, tag=name)
            nc.sync.dma_start(out=t,
                              in_=wfull[:, r0:r0 + nr, :].bitcast(dt))
            if dt == f8e4:
                tb = consts.tile(list(shape), bf16, tag=name + "_b")
                nc.vector.tensor_copy(tb, t)
                return tb
            return t

        # ---- gather indices: replicate 16-row wrap to the 8 gpsimd
        # stripes with a doubling ladder
        idxs_sb = consts.tile([128, DEG, NW], i16, tag="idxs")
        idxl_sb = consts.tile([128, NW], i16, tag="idxl")
        nc.gpsimd.iota(idxl_sb[0:16, :], pattern=[[16, NW]], base=0,
                       channel_multiplier=1)
        nc.sync.dma_start(out=idxs_sb[0:16, :, :], in_=bsec("idx_steps", i16))
        for k in (16, 32, 64):
            nc.sync.dma_start(out=idxs_sb[k:2 * k, :, :],
                              in_=idxs_sb[0:k, :, :])
            nc.sync.dma_start(out=idxl_sb[k:2 * k, :], in_=idxl_sb[0:k, :])
        idx_t = consts.tile([128, 1, NW], i16, tag="idx_t")

        def cload(sec, shape, dtype, tag, rows=slice(None)):
            t = consts.tile(shape, dtype, tag=tag)
            nc.sync.dma_start(out=t, in_=bsec(sec, dtype, rows))
            return t

        gl8_sb = cload("gl", [128, NB], mybir.dt.int8, "gl8")
        gl_sb = consts.tile([128, NB], f32, tag="gl")
        nc.vector.tensor_copy(gl_sb, gl8_sb)
        inv_sb = cload("invg", [G, 1], f32, "invg", rows=slice(0, G))
        if has_bias:
            blstm1_sb = cload("blstm1", [128, 4], f32, "blstm1")
            blstm2_sb = cload("blstm2", [128, 8], f32, "blstm2")
            b1bc_sb = gload("b1bc")
            b2bc_sb = gload("b2bc")
            bmu_sb = gload("bmu")
            bsig_sb = gload("bsig")

        wihT1_sb = gload("wihT1")
        whhT1_sb = gload("whhT1")
        wself1_sb = gload("wself1")
        wneigh1_sb = gload("wneigh1")
        wihT2_sb = gload("wihT2")
        whhT2_sb = gload("whhT2")
        wself2_sb = gload("wself2")
        wneigh2_sb = gload("wneigh2")
        wmu_sb = gload("wmu")
        wsig_sb = gload("wsig")

        # column-index iota row, for building per-block pool one-hots
        # (values 0..G-1 are exact in f32)
        iotaF = consts.tile([128, G], f32, tag="iotaF")
        nc.gpsimd.iota(iotaF, pattern=[[1, G]], base=0, channel_multiplier=0,
                       allow_small_or_imprecise_dtypes=True)

        # fp8 shard -> bf16 local table -> single AllGather (rank-major ==
        # original node order, so neighbor ids index the tables directly)
        with tc.tile_pool(name="cvt", bufs=2) as cvt:
            t8 = cvt.tile([128, NB * D_IN], f8e4, tag="t8")
            nc.sync.dma_start(out=t8, in_=bsec("feat8", f8e4))
            tb = cvt.tile([128, NB * D_IN], bf16, tag="tb")
            nc.vector.tensor_copy(tb, t8)
            for blk in range(NB):
                nc.sync.dma_start(
                    out=feat_bf[blk * 128:(blk + 1) * 128, :],
                    in_=tb[:, blk * 128:(blk + 1) * 128])
        nc.gpsimd.collective_compute(
            "AllGather", mybir.AluOpType.bypass, replica_groups=rg,
            ins=[feat_bf[:, :]], outs=[tab_feat[:, :]])

        snp = ctx.enter_context(tc.tile_pool(name="snp", bufs=3))

        # ================= Layer 1 =================
        with tc.tile_pool(name="st1", bufs=1) as st1, \
             tc.tile_pool(name="gt1", bufs=1) as gtp, \
             tc.tile_pool(name="xg1p", bufs=1) as xgp:
            hN1 = st1.tile([128, NLOC], bf16, tag="hN1")
            cN1 = st1.tile([128, NLOC], f32, tag="cN1")
            nc.vector.memset(hN1, 0.0)
            nc.vector.memset(cN1, 0.0)
            featT = st1.tile([128, 1, NLOC], bf16, tag="featT")
            nc.gpsimd.dma_gather(featT[:], feat_bf[:], idxl_sb[:, :],
                                 NLOC, NLOC, D_IN, transpose=True,
                                 single_packet=False)

            with tc.tile_pool(name="psl1", bufs=1, space="PSUM") as psl:
                with tc.For_i(0, DEG) as t:
                    nc.sync.dma_start(out=idx_t,
                                      in_=idxs_sb[:, ds(t, 1), :])
                    xg = xgp.tile([128, 1, NLOC], bf16, tag="xg1")
                    nc.gpsimd.dma_gather(
                        xg[:], tab_feat[:], idx_t[:, 0, :],
                        NLOC, NLOC, D_IN, transpose=True,
                        single_packet=False)
                    with tc.For_i(0, NLOC, 512) as go:
                        gsl = ds(go, 512)
                        ps = psl.tile([128, 4, 512], f32, tag="ps1")
                        for q in range(4):
                            o = ps[:, q, :]
                            nc.tensor.matmul(
                                o, wihT1_sb[:, q * 128:(q + 1) * 128],
                                xg[:, 0, gsl], start=True, stop=False)
                            nc.tensor.matmul(
                                o, whhT1_sb[:, q * 128:(q + 1) * 128],
                                hN1[:, gsl], start=False, stop=True)
                        ifo = gtp.tile([128, 3, 512], f32, tag="ifo1")
                        gg = gtp.tile([128, 1, 512], f32, tag="gg1")
                        if has_bias:
                            for q, fn in ((0, Sig), (1, Sig), (2, Sig)):
                                nc.scalar.activation(
                                    ifo[:, q, :], ps[:, q, :], fn,
                                    bias=blstm1_sb[:, q:q + 1])
                            nc.scalar.activation(gg[:, 0, :], ps[:, 3, :],
                                                 Tnh, bias=blstm1_sb[:, 3:4])
                        else:
                            nc.scalar.activation(ifo, ps[:, 0:3, :], Sig)
                            nc.scalar.activation(gg, ps[:, 3:4, :], Tnh)
                        t0 = gtp.tile([128, 512], f32, tag="t01")
                        nc.vector.tensor_mul(t0, ifo[:, 0, :], gg[:, 0, :])
                        nc.vector.tensor_mul(cN1[:, gsl], cN1[:, gsl],
                                             ifo[:, 1, :])
                        nc.vector.tensor_add(cN1[:, gsl], cN1[:, gsl], t0)
                        tch = gtp.tile([128, 512], f32, tag="tch1")
                        nc.scalar.activation(tch, cN1[:, gsl], Tnh)
                        nc.vector.tensor_mul(hN1[:, gsl], ifo[:, 2, :], tch)

            # self/neigh + relu -> h1_shard; then one AllGather. The node
            # slices are staged through fixed tiles so the matmul lhsT stays
            # register-offset-free inside the loop.
            with tc.tile_pool(name="psm1", bufs=2, space="PSUM") as psm, \
                 tc.tile_pool(name="tmp1", bufs=1) as tmpp:
                with tc.For_i(0, NLOC, 128) as bo:
                    tf = tmpp.tile([128, 128], bf16, tag="tf")
                    nc.sync.dma_start(out=tf, in_=featT[:, 0, ds(bo, 128)])
                    th = tmpp.tile([128, 128], bf16, tag="th")
                    nc.sync.dma_start(out=th, in_=hN1[:, ds(bo, 128)])
                    ps = psm.tile([128, D_FEAT], f32, tag="psm1")
                    nc.tensor.matmul(ps, tf, wself1_sb[:, :],
                                     start=True, stop=False)
                    nc.tensor.matmul(ps, th, wneigh1_sb[:, :],
                                     start=False, stop=True)
                    h1b = snp.tile([128, D_FEAT], bf16, tag="sn1b")
                    if has_bias:
                        tmp = snp.tile([128, D_FEAT], f32, tag="sn1t")
                        nc.vector.tensor_add(tmp, ps, b1bc_sb)
                        nc.scalar.activation(h1b, tmp, Rlu)
                    else:
                        nc.scalar.activation(h1b, ps, Rlu)
                    nc.sync.dma_start(out=h1_shard[ds(bo, 128), :], in_=h1b)
            nc.gpsimd.collective_compute(
                "AllGather", mybir.AluOpType.bypass, replica_groups=rg,
                ins=[h1_shard[:, :]], outs=[h1_full[:, :]])

        import concourse.mybir as _mb

        if stop_after == "l1":
            with tc.tile_pool(name="dbgp", bufs=2) as dbgp:
                for blk in range(N // 128):
                    dt_ = dbgp.tile([128, D_FEAT], bf16, tag="dbg")
                    nc.sync.dma_start(
                        out=dt_, in_=h1_full[blk * 128:(blk + 1) * 128, :])
                    nc.sync.dma_start(
                        out=dbg_h1[blk * 128:(blk + 1) * 128, :], in_=dt_)
        else:
            # ================= Layer 2 =================
            with tc.tile_pool(name="st2", bufs=1) as st2, \
                 tc.tile_pool(name="gt2", bufs=1) as gtp, \
                 tc.tile_pool(name="xg2p", bufs=1) as xgp:
                hN2 = st2.tile([128, 2, NLOC], bf16, tag="hN2")
                cN2 = st2.tile([128, 2, NLOC], f32, tag="cN2")
                nc.vector.memset(hN2, 0.0)
                nc.vector.memset(cN2, 0.0)
                h1T = st2.tile([128, 2, NLOC], bf16, tag="h1T")
                nc.gpsimd.dma_gather(h1T[:], h1_shard[:], idxl_sb[:, :],
                                     NLOC, NLOC, D_FEAT, transpose=True,
                                     single_packet=False)

                with tc.tile_pool(name="psl2", bufs=1, space="PSUM") as psl:
                    with tc.For_i(0, DEG) as t:
                        nc.sync.dma_start(out=idx_t,
                                          in_=idxs_sb[:, ds(t, 1), :])
                        xg = xgp.tile([128, 2, NLOC], bf16, tag="xg2")
                        nc.gpsimd.dma_gather(
                            xg[:], h1_full[:], idx_t[:, 0, :],
                            NLOC, NLOC, D_FEAT, transpose=True,
                            single_packet=False)
                        with tc.For_i(0, NLOC, L2G) as so:
                            nsl = ds(so, L2G)
                            ps = psl.tile([128, 8, L2G], f32, tag="ps2")
                            for q in range(4):
                                for mb in range(2):
                                    o = ps[:, 2 * q + mb, :]
                                    wsl = slice(q * 256 + mb * 128,
                                                q * 256 + (mb + 1) * 128)
                                    for kb in range(2):
                                        nc.tensor.matmul(
                                            o, wihT2_sb[:, kb, wsl],
                                            xg[:, kb, nsl],
                                            start=(kb == 0), stop=False)
                                    for kb in range(2):
                                        nc.tensor.matmul(
                                            o, whhT2_sb[:, kb, wsl],
                                            hN2[:, kb, nsl],
                                            start=False, stop=(kb == 1))
                            ifo = gtp.tile([128, 6, L2G], f32, tag="ifo2")
                            gg2 = gtp.tile([128, 2, L2G], f32, tag="gg2")
                            if has_bias:
                                for q in range(3):
                                    for mb in range(2):
                                        nc.scalar.activation(
                                            ifo[:, 2 * q + mb, :],
                                            ps[:, 2 * q + mb, :], Sig,
                                            bias=blstm2_sb[:, 2 * q + mb:
                                                           2 * q + mb + 1])
                                for mb in range(2):
                                    nc.scalar.activation(
                                        gg2[:, mb, :], ps[:, 6 + mb, :], Tnh,
                                        bias=blstm2_sb[:, 6 + mb:7 + mb])
                            else:
                                nc.scalar.activation(ifo, ps[:, 0:6, :], Sig)
                                nc.scalar.activation(gg2, ps[:, 6:8, :], Tnh)
                            t0 = gtp.tile([128, 2, L2G], f32, tag="t02")
                            nc.vector.tensor_mul(t0, ifo[:, 0:2, :], gg2)
                            nc.vector.tensor_mul(cN2[:, :, nsl],
                                                 cN2[:, :, nsl],
                                                 ifo[:, 2:4, :])
                            nc.vector.tensor_add(cN2[:, :, nsl],
                                                 cN2[:, :, nsl], t0)
                            tch = gtp.tile([128, 2, L2G], f32, tag="tch2")
                            nc.scalar.activation(tch, cN2[:, :, nsl], Tnh)
                            nc.vector.tensor_mul(hN2[:, :, nsl],
                                                 ifo[:, 4:6, :], tch)

                # L2 self/neigh + pooling, as a hardware loop: node slices
                # staged into fixed tiles (matmul lhsT can't take register
                # offsets); the pool one-hots are pre-built in a 128-padded
                # bank so ds(node_offset, G) indexes them; the pool PSUM
                # accumulation group is opened/closed by K=1 zero matmuls.
                ohp = st2.tile([128, NB * 128], bf16, tag="ohp")
                for blk in range(NB):
                    nc.vector.tensor_scalar(
                        ohp[:, blk * 128:blk * 128 + G], iotaF,
                        gl_sb[:, blk:blk + 1], None, _mb.AluOpType.is_equal)
                with tc.tile_pool(name="psm2", bufs=2, space="PSUM") as psm, \
                     tc.tile_pool(name="pspool", bufs=2, space="PSUM") as psp, \
                     tc.tile_pool(name="pshead", bufs=2, space="PSUM") as psh, \
                     tc.tile_pool(name="tmp2", bufs=1) as tmpp:
                    pool_ps = [psp.tile([128, G], f32, tag=f"pool{mh}",
                                        name=f"pool_ps{mh}")
                               for mh in range(2)]
                    zrow = snp.tile([1, 128], bf16, tag="zrow")
                    nc.vector.memset(zrow, 0.0)
                    for mh in range(2):
                        nc.tensor.matmul(pool_ps[mh], zrow[0:1, :],
                                         zrow[0:1, 0:G],
                                         start=True, stop=False,
                                         skip_group_check=True)
                    with tc.For_i(0, NLOC, 128) as bo:
                        t1 = tmpp.tile([128, 2, 128], bf16, tag="t1")
                        nc.sync.dma_start(out=t1, in_=h1T[:, :, ds(bo, 128)])
                        t2 = tmpp.tile([128, 2, 128], bf16, tag="t2")
                        nc.sync.dma_start(out=t2, in_=hN2[:, :, ds(bo, 128)])
                        ps = psm.tile([128, D_FEAT], f32, tag="psm2")
                        for kb in range(2):
                            nc.tensor.matmul(ps, t1[:, kb, :],
                                             wself2_sb[:, kb, :],
                                             start=(kb == 0), stop=False)
                        for kb in range(2):
                            nc.tensor.matmul(ps, t2[:, kb, :],
                                             wneigh2_sb[:, kb, :],
                                             start=False, stop=(kb == 1))
                        h2sb = snp.tile([128, D_FEAT], bf16, tag="h2sb")
                        if has_bias:
                            nc.vector.tensor_add(h2sb, ps, b2bc_sb)
                        else:
                            nc.vector.tensor_copy(h2sb, ps)
                        if stop_after == "l2":
                            nc.sync.dma_start(out=dbg_h2[ds(bo, 128), :],
                                              in_=h2sb)
                        for mh in range(2):
                            nc.tensor.matmul(
                                pool_ps[mh], h2sb[:, mh * 128:(mh + 1) * 128],
                                ohp[:, ds(bo, G)],
                                start=False, stop=False,
                                skip_group_check=True)
                    for mh in range(2):
                        nc.tensor.matmul(pool_ps[mh], zrow[0:1, :],
                                         zrow[0:1, 0:G],
                                         start=False, stop=True,
                                         skip_group_check=True)
                    prcp = snp.tile([128, 2, G], f32, tag="prcp")
                    for mh in range(2):
                        nc.vector.tensor_copy(prcp[:, mh, :], pool_ps[mh])
                    nc.sync.dma_start(out=pr_in[:, :, :], in_=prcp)
                    nc.gpsimd.collective_compute(
                        "AllReduce", _mb.AluOpType.add, replica_groups=rg,
                        ins=[pr_in[:]], outs=[pr_out[:]])
                    prx = snp.tile([128, 2, G], f32, tag="prx")
                    nc.sync.dma_start(out=prx, in_=pr_out[:, :, :])
                    if stop_after == "l2":
                        nc.sync.dma_start(out=dbg_pr[:, :, :], in_=prx)
                    else:
                        prb = snp.tile([128, 2, G], bf16, tag="prb")
                        nc.vector.tensor_copy(prb, prx)
                        for hd, (wsb, bsb) in enumerate((
                                (wmu_sb, "bmu"), (wsig_sb, "bsig"))):
                            ph = psh.tile([G, D_REP], f32, tag="ph")
                            for kb in range(2):
                                nc.tensor.matmul(ph, prb[:, kb, :],
                                                 wsb[:, kb, :],
                                                 start=(kb == 0),
                                                 stop=(kb == 1))
                            # mean = sum * (1/cnt), per graph row
                            ores = snp.tile([G, D_REP], bf16, tag="ores")
                            if has_bias:
                                phs = snp.tile([G, D_REP], f32, tag="phs")
                                nc.scalar.activation(phs, ph, Cpy,
                                                     scale=inv_sb[:, 0:1])
                                nc.vector.tensor_add(
                                    ores, phs,
                                    bmu_sb if bsb == "bmu" else bsig_sb)
                            else:
                                nc.scalar.activation(ores, ph, Cpy,
                                                     scale=inv_sb[:, 0:1])
                            nc.sync.dma_start(out=out2[hd, :, :], in_=ores)


    nc.compile()
    return nc


def make_inmaps(inputs, N, DEG, G, NCORE):
    """Host-side preprocessing: pack the full inputs into per-core blobs."""
    hb = _has_bias(inputs)
    L = _layout(N, DEG, G, NCORE, hb)
    NLOC, NB, NW, WPC = L["NLOC"], L["NB"], L["NW"], L["WPC"]
    cols, W = L["cols"], L["W"]

    feat = np.asarray(inputs["in_feat"], dtype=F32)
    nbr = np.asarray(inputs["neighbors"], dtype=np.int64)
    n2g = np.asarray(inputs["node2graph"], dtype=np.int64)

    def A(name):
        return np.asarray(inputs[name], dtype=F32)

    def wrap16(ids):
        # ids [n] -> [16, n//16] int16 wrap -> [128, n//8 bytes] row split
        # matching the on-device stripe-replication DMA's flat streaming.
        w = np.ascontiguousarray(ids.reshape(-1, 16).T.astype(np.int16))
        return w.view(np.uint8).reshape(128, -1)

    def gperm(wT, ngate_el):
        # reorder the gate axis (pytorch i,f,g,o -> i,f,o,g) of a packed
        # [..., 4*ngate_el] last axis
        s = wT.shape
        w = wT.reshape(s[:-1] + (4, ngate_el))[..., GPERM, :]
        return np.ascontiguousarray(w.reshape(s))

    full = {
        "wihT1": gperm(np.ascontiguousarray(A("w_ih1").T), 128).astype(F8),
        "whhT1": gperm(np.ascontiguousarray(A("w_hh1").T), 128).astype(F8),
        "wself1": A("w_self1").astype(BF),
        "wneigh1": A("w_neigh1").astype(BF),
        "b1bc": np.tile(A("b1")[None, :], (128, 1)).astype(F32),
        "wihT2": gperm(np.ascontiguousarray(
            A("w_ih2").T.reshape(2, 128, 4 * D_FEAT).transpose(1, 0, 2)),
            D_FEAT).astype(F8),
        "whhT2": gperm(np.ascontiguousarray(
            A("w_hh2").T.reshape(2, 128, 4 * D_FEAT).transpose(1, 0, 2)),
            D_FEAT).astype(F8),
        "wself2": np.ascontiguousarray(
            A("w_self2").reshape(2, 128, D_FEAT).transpose(1, 0, 2)).astype(BF),
        "wneigh2": np.ascontiguousarray(
            A("w_neigh2").reshape(2, 128, D_FEAT).transpose(1, 0, 2)).astype(BF),
        "b2bc": np.tile(A("b2")[None, :], (128, 1)).astype(F32),
        "wmu": np.ascontiguousarray(
            A("w_mu").reshape(2, 128, D_REP).transpose(1, 0, 2)).astype(BF),
        "wsig": np.ascontiguousarray(
            A("w_sigma").reshape(2, 128, D_REP).transpose(1, 0, 2)).astype(BF),
        "bmu": np.tile(A("b_mu")[None, :], (G, 1)).astype(F32),
        "bsig": np.tile(A("b_sigma")[None, :], (G, 1)).astype(F32),
    }

    blstm1 = np.ascontiguousarray(
        A("b_lstm1").reshape(4, 128)[GPERM].T).astype(F32)      # [128, 4]
    blstm2 = np.ascontiguousarray(
        A("b_lstm2").reshape(4, 2, 128)[GPERM].transpose(2, 0, 1)
        .reshape(128, 8)).astype(F32)

    cnt = np.bincount(n2g, minlength=G).astype(F32)
    inv = (1.0 / np.maximum(cnt, 1.0))[:, None].astype(F32)     # [G, 1]

    def put(bb, sec, rows, arr8):
        c0, w = cols[sec]
        bb[rows, c0 * 4:c0 * 4 + arr8.shape[-1]] = arr8

    in_maps = []
    for c in range(NCORE):
        base = c * NLOC
        blob = np.zeros((128, W), np.uint32)
        bb = blob.view(np.uint8).reshape(128, W * 4)

        f8 = feat[base:base + NLOC].astype(F8)
        f8 = f8.reshape(NB, 128, D_IN).transpose(1, 0, 2).reshape(128, NB * D_IN)
        put(bb, "feat8", slice(None), f8.view(np.uint8))

        ids = nbr[base:base + NLOC, :]  # [NLOC, DEG]
        w16 = np.ascontiguousarray(
            ids.reshape(NW, 16, DEG).transpose(1, 2, 0).astype(np.int16))
        steps8 = w16.view(np.uint8).reshape(128, -1)
        put(bb, "idx_steps", slice(None), steps8)
        if hb:
            put(bb, "blstm1", slice(None), blstm1.view(np.uint8))
            put(bb, "blstm2", slice(None), blstm2.view(np.uint8))
        gl = np.ascontiguousarray(
            n2g[base:base + NLOC].reshape(NB, 128).T.astype(np.int8))
        put(bb, "gl", slice(None), gl.view(np.uint8))
        put(bb, "invg", slice(0, G), inv.view(np.uint8))

        slab = b"".join(
            np.ascontiguousarray(
                full[name][(full[name].shape[0] // NCORE) * c:
                           (full[name].shape[0] // NCORE) * (c + 1)]
            ).tobytes()
            for name, _, _ in _wpack(hb))
        put(bb, "wslab", slice(None),
            np.frombuffer(slab, np.uint8).reshape(128, WPC // 128))

        in_maps.append({"blob": blob})
    return in_maps


def _has_bias(inputs):
    return any(
        np.any(np.asarray(inputs[k]))
        for k in ("b_lstm1", "b_lstm2", "b1", "b2", "b_mu", "b_sigma"))


_PROG = None
_PROG_BIAS = None


def kernel(**inputs):
    global _PROG, _PROG_BIAS
    from concourse.bass_utils import run_bass_kernel_spmd

    cfg = FULL
    hb = _has_bias(inputs)
    if _PROG is None or _PROG_BIAS != hb:
        _PROG = build_program(**cfg, has_bias=hb)
        _PROG_BIAS = hb
    in_maps = make_inmaps(inputs, **cfg)
    res = run_bass_kernel_spmd(_PROG, in_maps, core_ids=list(range(cfg["NCORE"])))
    out = np.asarray(res.results[0]["out2"], dtype=np.float32)
    return (out[0], out[1])
